# revision 2
# baseline (speedup 1.0000x reference)
"""KNN top-16 kernel for Trainium2 (8 NeuronCores, SPMD).

Strategy (matches the corpus-sharding hint):
  - X_train (65536 rows) is sharded 8192 rows per core; X_test (4096 rows) is
    replicated.
  - Each core computes s[q, c] = 2 * <X_test[q], X_train[c]> - ||X_train[c]||^2
    for its shard.  Ranking by descending s is identical to ranking by
    ascending squared euclidean distance (the per-query ||X_test[q]||^2 term
    is constant along the candidate axis and drops out).
  - The matmul runs in true fp32 on the PE array (4-pass H/L fp32 mode).  The
    -||X_train||^2 term is folded into the same PSUM accumulation group as an
    extra bf16 matmul: the fp32 value is split exactly into three bf16 addends
    (8+8+8 mantissa bits), placed on three rows of a zero-padded [128, C]
    operand, and contracted against a ones-column weight matrix.  This costs
    one 1-cycle/row pass instead of a 4-cycle/row fp32 pass and keeps full
    fp32 accuracy.
  - Per query tile of 128 (queries on PSUM partitions), scores are evacuated
    PSUM->SBUF on the scalar engine, then the vector engine extracts the top-8
    values + indices of each 1024-wide candidate chunk (hardware Max8 /
    MaxIndex instructions).  8 chunks * 8 = 64 candidates per (query, core).
  - The union of per-chunk top-8 lists contains the true global top-16 unless
    some 1024-candidate chunk holds >= 9 of the 16 global nearest neighbours
    (probability ~1e-7 over 4096 iid gaussian queries).
  - Host gathers 8 cores * 64 = 512 candidates per query and selects the
    final top-16 (ties broken by lower index, matching jax.lax.top_k).
"""

import numpy as np
import ml_dtypes

N_CORES = 8
NQ = 4096          # queries (X_test rows)
NTRAIN = 65536     # corpus (X_train rows)
KDIM = 256         # feature dim
SHARD = NTRAIN // N_CORES     # 8192 candidates per core
P = 128
NQT = NQ // P                 # 32 query tiles
CT = 512                      # matmul free-dim / PSUM bank
NCT = SHARD // CT             # 16 candidate tiles
CHUNK = 1024                  # top-8 chunk width
NCHUNK = SHARD // CHUNK       # 8 chunks -> 64 candidates/query/core
KC = KDIM // P                # 2 contraction chunks
TOPK = 16

_CACHE = {}


def _build_program(nqt=NQT):
    import concourse.mybir as mybir
    import concourse.tile as tile
    from concourse import bacc

    NQT = nqt  # noqa: N806 — allow scaled-down builds for simulation
    NQ = NQT * P  # noqa: N806

    nc = bacc.Bacc(
        "TRN2", target_bir_lowering=False, debug=False, enable_asserts=False
    )
    f32 = mybir.dt.float32
    bf16 = mybir.dt.bfloat16
    u32 = mybir.dt.uint32

    lhsT = nc.dram_tensor("lhsT", [P, KC, NQ], f32, kind="ExternalInput").ap()
    rhs = nc.dram_tensor("rhs", [P, KC, SHARD], f32, kind="ExternalInput").ap()
    tsp = nc.dram_tensor("tsp", [P, SHARD], bf16, kind="ExternalInput").ap()
    ones = nc.dram_tensor("ones", [P, P], bf16, kind="ExternalInput").ap()
    ovals = nc.dram_tensor("ovals", [NQT, P, 64], f32, kind="ExternalOutput").ap()
    oidx = nc.dram_tensor("oidx", [NQT, P, 64], u32, kind="ExternalOutput").ap()

    with tile.TileContext(nc) as tc:
        with (
            tc.tile_pool(name="const", bufs=1) as cpool,
            tc.tile_pool(name="scores", bufs=2) as spool,
            tc.tile_pool(name="outs", bufs=2) as opool,
            tc.tile_pool(name="psum", bufs=8, space="PSUM") as ppool,
        ):
            lhsT_sb = cpool.tile([P, KC, NQ], f32)
            rhs_sb = cpool.tile([P, KC, SHARD], f32)
            tsp_sb = cpool.tile([P, SHARD], bf16)
            ones_sb = cpool.tile([P, P], bf16)
            nc.sync.dma_start(ones_sb[:], ones[:])
            nc.sync.dma_start(tsp_sb[:], tsp[:])
            for kc in range(KC):
                nc.sync.dma_start(lhsT_sb[:, kc], lhsT[:, kc])
                half = SHARD // 2
                for h in range(2):
                    nc.sync.dma_start(
                        rhs_sb[:, kc, h * half : (h + 1) * half],
                        rhs[:, kc, h * half : (h + 1) * half],
                    )

            for qt in range(NQT):
                scores = spool.tile([P, SHARD], f32, tag="scores")
                for ct in range(NCT):
                    pt = ppool.tile([P, CT], f32, tag="ps")
                    csl = slice(ct * CT, (ct + 1) * CT)
                    nc.tensor.matmul(
                        pt[:], ones_sb[:], tsp_sb[:, csl], start=True, stop=False
                    )
                    for kc in range(KC):
                        nc.tensor.matmul(
                            pt[:],
                            lhsT_sb[:, kc, qt * P : (qt + 1) * P],
                            rhs_sb[:, kc, csl],
                            start=False,
                            stop=(kc == KC - 1),
                        )
                    nc.scalar.copy(scores[:, csl], pt[:])
                vals = opool.tile([P, 64], f32, tag="vals")
                idxs = opool.tile([P, 64], u32, tag="idxs")
                for ch in range(NCHUNK):
                    ssl = scores[:, ch * CHUNK : (ch + 1) * CHUNK]
                    osl = slice(ch * 8, (ch + 1) * 8)
                    nc.vector.max(out=vals[:, osl], in_=ssl)
                    nc.vector.max_index(
                        out=idxs[:, osl], in_max=vals[:, osl], in_values=ssl
                    )
                nc.sync.dma_start(ovals[qt], vals[:])
                nc.sync.dma_start(oidx[qt], idxs[:])

    nc.compile()
    return nc


def _split_bf16x3(v):
    """Split fp32 vector v exactly into three bf16 addends (v = a + b + c)."""
    a = v.astype(ml_dtypes.bfloat16)
    r1 = (v - a.astype(np.float32)).astype(np.float32)
    b = r1.astype(ml_dtypes.bfloat16)
    r2 = (r1 - b.astype(np.float32)).astype(np.float32)
    c = r2.astype(ml_dtypes.bfloat16)
    return a, b, c


def _prep_inputs(X_train, X_test):
    X_train = np.asarray(X_train, dtype=np.float32)
    X_test = np.asarray(X_test, dtype=np.float32)

    lhsT_np = np.ascontiguousarray(
        X_test.T.reshape(KC, P, NQ).transpose(1, 0, 2)
    )
    ones_np = np.zeros((P, P), dtype=ml_dtypes.bfloat16)
    ones_np[0:3] = 1.0

    in_maps = []
    for core in range(N_CORES):
        shard = X_train[core * SHARD : (core + 1) * SHARD]
        rhs_np = np.ascontiguousarray(
            (2.0 * shard.T).reshape(KC, P, SHARD).transpose(1, 0, 2)
        )
        neg_sq = -np.sum(shard * shard, axis=1, dtype=np.float32)
        a, b, c = _split_bf16x3(neg_sq.astype(np.float32))
        tsp_np = np.zeros((P, SHARD), dtype=ml_dtypes.bfloat16)
        tsp_np[0], tsp_np[1], tsp_np[2] = a, b, c
        in_maps.append(
            {"lhsT": lhsT_np, "rhs": rhs_np, "tsp": tsp_np, "ones": ones_np}
        )
    return in_maps


def _merge_topk(results):
    """Merge per-core per-chunk top-8 candidate lists into global top-16."""
    all_vals = np.empty((NQ, N_CORES * NCHUNK * 8), dtype=np.float32)
    all_gidx = np.empty((NQ, N_CORES * NCHUNK * 8), dtype=np.int64)
    chunk_base = (np.arange(NCHUNK * 8, dtype=np.int64) // 8) * CHUNK
    for core in range(N_CORES):
        vals = results[core]["ovals"].reshape(NQ, NCHUNK * 8)
        idxs = results[core]["oidx"].reshape(NQ, NCHUNK * 8).astype(np.int64)
        gidx = idxs + chunk_base[None, :] + core * SHARD
        sl = slice(core * NCHUNK * 8, (core + 1) * NCHUNK * 8)
        all_vals[:, sl] = vals
        all_gidx[:, sl] = gidx

    # Order candidates by ascending global index, then stable-sort by
    # descending score: ties (equal distance) resolve to the lower index,
    # matching jax.lax.top_k.
    order_idx = np.argsort(all_gidx, axis=1, kind="stable")
    v = np.take_along_axis(all_vals, order_idx, axis=1)
    g = np.take_along_axis(all_gidx, order_idx, axis=1)
    order_val = np.argsort(-v, axis=1, kind="stable")[:, :TOPK]
    top_gidx = np.take_along_axis(g, order_val, axis=1)
    return top_gidx.astype(np.int32)


def _get_nc():
    if "nc" not in _CACHE:
        _CACHE["nc"] = _build_program()
    return _CACHE["nc"]


def kernel(X_train, X_test):
    from concourse.bass_utils import run_bass_kernel_spmd

    nc = _get_nc()
    in_maps = _prep_inputs(X_train, X_test)
    res = run_bass_kernel_spmd(nc, in_maps, core_ids=list(range(N_CORES)))
    return _merge_topk(res.results)


# revision 3
# speedup vs baseline: 1632.8846x; 1632.8846x over previous
"""KNN top-16 kernel for Trainium2 (8 NeuronCores, SPMD).

Strategy (matches the corpus-sharding hint):
  - X_train (65536 rows) is sharded 8192 rows per core; X_test (4096 rows) is
    replicated.
  - Each core computes s[q, c] = 2 * <X_test[q], X_train[c]> - ||X_train[c]||^2
    for its shard.  Ranking by descending s is identical to ranking by
    ascending squared euclidean distance (the per-query ||X_test[q]||^2 term
    is constant along the candidate axis and drops out).
  - The matmul runs in true fp32 on the PE array (4-pass H/L fp32 mode).  The
    -||X_train||^2 term is folded into the same PSUM accumulation group as an
    extra bf16 matmul: the fp32 value is split exactly into three bf16 addends
    (8+8+8 mantissa bits), placed on three rows of a zero-padded [128, C]
    operand, and contracted against a ones-column weight matrix.  This costs
    one 1-cycle/row pass instead of a 4-cycle/row fp32 pass and keeps full
    fp32 accuracy.
  - Per query tile of 128 (queries on PSUM partitions), scores are evacuated
    PSUM->SBUF on the scalar engine, then the vector engine extracts the top-8
    values + indices of each 1024-wide candidate chunk (hardware Max8 /
    MaxIndex instructions).  8 chunks * 8 = 64 candidates per (query, core).
  - The union of per-chunk top-8 lists contains the true global top-16 unless
    some 1024-candidate chunk holds >= 9 of the 16 global nearest neighbours
    (probability ~1e-7 over 4096 iid gaussian queries).
  - Host gathers 8 cores * 64 = 512 candidates per query and selects the
    final top-16 (ties broken by lower index, matching jax.lax.top_k).
"""

import numpy as np
import ml_dtypes

N_CORES = 8
NQ = 4096          # queries (X_test rows)
NTRAIN = 65536     # corpus (X_train rows)
KDIM = 256         # feature dim
SHARD = NTRAIN // N_CORES     # 8192 candidates per core
P = 128
NQT = NQ // P                 # 32 query tiles
CT = 512                      # matmul free-dim / PSUM bank
NCT = SHARD // CT             # 16 candidate tiles
CHUNK = 1024                  # top-8 chunk width
NCHUNK = SHARD // CHUNK       # 8 chunks -> 64 candidates/query/core
KC = KDIM // P                # 2 contraction chunks
TOPK = 16

_CACHE = {}


def _build_program(nqt=NQT):
    import concourse.mybir as mybir
    import concourse.tile as tile
    from concourse import bacc

    NQT = nqt  # noqa: N806 — allow scaled-down builds for simulation
    NQ = NQT * P  # noqa: N806

    nc = bacc.Bacc(
        "TRN2", target_bir_lowering=False, debug=False, enable_asserts=False
    )
    f32 = mybir.dt.float32
    bf16 = mybir.dt.bfloat16
    u32 = mybir.dt.uint32

    lhsT = nc.dram_tensor("lhsT", [P, KC, NQ], f32, kind="ExternalInput").ap()
    rhs = nc.dram_tensor("rhs", [P, KC, SHARD], f32, kind="ExternalInput").ap()
    tsp = nc.dram_tensor("tsp", [P, SHARD], bf16, kind="ExternalInput").ap()
    ones = nc.dram_tensor("ones", [P, P], bf16, kind="ExternalInput").ap()
    ovals = nc.dram_tensor("ovals", [NQT, P, 64], f32, kind="ExternalOutput").ap()
    oidx = nc.dram_tensor("oidx", [NQT, P, 64], u32, kind="ExternalOutput").ap()

    with tile.TileContext(nc) as tc:
        with (
            tc.tile_pool(name="const", bufs=1) as cpool,
            tc.tile_pool(name="scores", bufs=2) as spool,
            tc.tile_pool(name="outs", bufs=2) as opool,
            tc.tile_pool(name="psum", bufs=8, space="PSUM") as ppool,
        ):
            lhsT_sb = cpool.tile([P, KC, NQ], f32)
            rhs_sb = cpool.tile([P, KC, SHARD], f32)
            tsp_sb = cpool.tile([P, SHARD], bf16)
            ones_sb = cpool.tile([P, P], bf16)
            nc.sync.dma_start(ones_sb[:], ones[:])
            nc.sync.dma_start(tsp_sb[:], tsp[:])
            for kc in range(KC):
                nc.sync.dma_start(lhsT_sb[:, kc], lhsT[:, kc])
                half = SHARD // 2
                for h in range(2):
                    nc.sync.dma_start(
                        rhs_sb[:, kc, h * half : (h + 1) * half],
                        rhs[:, kc, h * half : (h + 1) * half],
                    )

            for qt in range(NQT):
                scores = spool.tile([P, SHARD], f32, tag="scores")
                for ct in range(NCT):
                    pt = ppool.tile([P, CT], f32, tag="ps")
                    csl = slice(ct * CT, (ct + 1) * CT)
                    nc.tensor.matmul(
                        pt[:], ones_sb[:], tsp_sb[:, csl], start=True, stop=False
                    )
                    for kc in range(KC):
                        nc.tensor.matmul(
                            pt[:],
                            lhsT_sb[:, kc, qt * P : (qt + 1) * P],
                            rhs_sb[:, kc, csl],
                            start=False,
                            stop=(kc == KC - 1),
                        )
                    nc.scalar.copy(scores[:, csl], pt[:])
                vals = opool.tile([P, 64], f32, tag="vals")
                idxs = opool.tile([P, 64], u32, tag="idxs")
                for ch in range(NCHUNK):
                    ssl = scores[:, ch * CHUNK : (ch + 1) * CHUNK]
                    osl = slice(ch * 8, (ch + 1) * 8)
                    nc.vector.max(out=vals[:, osl], in_=ssl)
                    nc.vector.max_index(
                        out=idxs[:, osl], in_max=vals[:, osl], in_values=ssl
                    )
                nc.sync.dma_start(ovals[qt], vals[:])
                nc.sync.dma_start(oidx[qt], idxs[:])

    nc.compile()
    return nc


def _split_bf16x3(v):
    """Split fp32 vector v exactly into three bf16 addends (v = a + b + c)."""
    a = v.astype(ml_dtypes.bfloat16)
    r1 = (v - a.astype(np.float32)).astype(np.float32)
    b = r1.astype(ml_dtypes.bfloat16)
    r2 = (r1 - b.astype(np.float32)).astype(np.float32)
    c = r2.astype(ml_dtypes.bfloat16)
    return a, b, c


def _prep_inputs(X_train, X_test):
    X_train = np.asarray(X_train, dtype=np.float32)
    X_test = np.asarray(X_test, dtype=np.float32)

    lhsT_np = np.ascontiguousarray(
        X_test.T.reshape(KC, P, NQ).transpose(1, 0, 2)
    )
    ones_np = np.zeros((P, P), dtype=ml_dtypes.bfloat16)
    ones_np[0:3] = 1.0

    in_maps = []
    for core in range(N_CORES):
        shard = X_train[core * SHARD : (core + 1) * SHARD]
        rhs_np = np.ascontiguousarray(
            (2.0 * shard.T).reshape(KC, P, SHARD).transpose(1, 0, 2)
        )
        neg_sq = -np.sum(shard * shard, axis=1, dtype=np.float32)
        a, b, c = _split_bf16x3(neg_sq.astype(np.float32))
        tsp_np = np.zeros((P, SHARD), dtype=ml_dtypes.bfloat16)
        tsp_np[0], tsp_np[1], tsp_np[2] = a, b, c
        in_maps.append(
            {"lhsT": lhsT_np, "rhs": rhs_np, "tsp": tsp_np, "ones": ones_np}
        )
    return in_maps


def _merge_topk(results):
    """Merge per-core per-chunk top-8 candidate lists into global top-16."""
    all_vals = np.empty((NQ, N_CORES * NCHUNK * 8), dtype=np.float32)
    all_gidx = np.empty((NQ, N_CORES * NCHUNK * 8), dtype=np.int64)
    chunk_base = (np.arange(NCHUNK * 8, dtype=np.int64) // 8) * CHUNK
    for core in range(N_CORES):
        vals = results[core]["ovals"].reshape(NQ, NCHUNK * 8)
        idxs = results[core]["oidx"].reshape(NQ, NCHUNK * 8).astype(np.int64)
        gidx = idxs + chunk_base[None, :] + core * SHARD
        sl = slice(core * NCHUNK * 8, (core + 1) * NCHUNK * 8)
        all_vals[:, sl] = vals
        all_gidx[:, sl] = gidx

    # Order candidates by ascending global index, then stable-sort by
    # descending score: ties (equal distance) resolve to the lower index,
    # matching jax.lax.top_k.
    order_idx = np.argsort(all_gidx, axis=1, kind="stable")
    v = np.take_along_axis(all_vals, order_idx, axis=1)
    g = np.take_along_axis(all_gidx, order_idx, axis=1)
    order_val = np.argsort(-v, axis=1, kind="stable")[:, :TOPK]
    top_gidx = np.take_along_axis(g, order_val, axis=1)
    return top_gidx.astype(np.int32)


def _get_nc():
    if "nc" not in _CACHE:
        _CACHE["nc"] = _build_program()
    return _CACHE["nc"]


def kernel(X_train, X_test):
    from concourse.bass_utils import run_bass_kernel_spmd

    nc = _get_nc()
    in_maps = _prep_inputs(X_train, X_test)
    last_err = None
    for _attempt in range(3):
        try:
            res = run_bass_kernel_spmd(
                nc, in_maps, core_ids=list(range(N_CORES))
            )
            break
        except Exception as e:  # transient NRT exec errors — retry
            last_err = e
    else:
        raise last_err
    return _merge_topk(res.results)
